# revision 3
# baseline (speedup 1.0000x reference)
"""Trainium2 Bass kernel for nn_LoRALinear (DoRA-style LoRA linear).

Reference math (per problem):
    base = x @ W^T
    lora = sc * (x @ A^T) @ B^T          (sc = 2.0)
    w_eff = W + sc * (B @ A)
    s = magnitude / ||w_eff||_row         (row norm over in_dim)
    out = base + (s - 1) * base + s * lora
        = s * (base + lora)
        = x @ (s[:, None] * w_eff)^T

The whole op collapses to one dense matmul with a derived weight. The
derived weight is tiny (1024x1024) so it is computed on the host in fp32
and shipped per-core as bf16; the device does nothing but the big GEMM.

Sharding: data-parallel over batch*seq across 8 cores (4096 tokens each).

Layouts (host-prepared so the device never transposes):
  xt:  per core, token tile m (128 tokens) stored as [128 q, 8*128 (k,t)]
       with q = d_in within k-strip, t = token within tile. Each k-slice
       [:, k*128:(k+1)*128] is directly the stationary lhsT of a matmul.
  wp:  packed weight [128 q, 8*1024 (k,n)]: wp[q, k*1024+n] =
       wT[k*128+q, n] — one 2 MB DMA, each (k,h) slice is a moving rhs.
  out: [tokens, d_out] bf16, PSUM fp32 accumulate, converted on drain.

Per-core schedule:
  ~24 junk matmuls on a memset tile warm the PE p-state while the
  weight/x0 DMAs land (the PE runs at half speed until ~3us of
  continuous busy, and the DMAs need ~6us after the kernel preamble)
  main loop over 32 token tiles:
    DMA xt tile (256 KB, packets spread over all 16 DMA engines)
    2 psum halves x 8 k matmuls (bf16, 512-row moving dim, 1 cyc/row)
    drain half 0 on ACT, half 1 on DVE (psum fp32 -> sbuf bf16)
    DMA each half out as soon as its drain lands (shorter last-tile tail)
"""

import os
import numpy as np
import ml_dtypes
from contextlib import ExitStack

import concourse.bass as bass
import concourse.mybir as mybir
import concourse.tile as tile
from concourse import bacc
from concourse.bass import ts
from concourse.bass_utils import run_bass_kernel_spmd

N_CORES = 8
B, S, D_IN, D_OUT, R = 4, 8192, 1024, 1024, 16
SCALING = 32.0 / 16.0
M_TOT = B * S                 # 32768 tokens
M_CORE = M_TOT // N_CORES     # 4096 tokens per core
P = 128
M_TILES = M_CORE // P         # 32
K_TILES = D_IN // P           # 8
NH = D_OUT // 512             # 2 n-halves of 512
N_WARMUP = 24                 # junk matmuls to ramp the PE p-state
F32 = mybir.dt.float32
BF16 = mybir.dt.bfloat16
NP_BF16 = ml_dtypes.bfloat16


def _kernel_body(ctx: ExitStack, tc: "tile.TileContext", xt, wp, out):
    nc = tc.nc
    const_pool = ctx.enter_context(tc.tile_pool(name="const", bufs=1))
    w_pool = ctx.enter_context(tc.tile_pool(name="w", bufs=1))
    x_pool = ctx.enter_context(tc.tile_pool(name="x", bufs=4))
    o_pool = ctx.enter_context(tc.tile_pool(name="o", bufs=4))
    ps_pool = ctx.enter_context(tc.tile_pool(name="ps", bufs=8, space="PSUM"))

    # first x tile requested before the weights: the first matmul needs it
    # and the weight is 8x bigger
    xt0 = x_pool.tile([P, D_IN], BF16, tag="xt", name="xt0")
    nc.sync.dma_start(xt0[:], xt[ts(0, P), :])

    w_sb = w_pool.tile([P, K_TILES * D_OUT], BF16, tag="w", name="w")
    nc.sync.dma_start(w_sb[:], wp[:, :])

    # PE p-state warmup: junk matmuls with no DMA dependency fill the PE
    # while the preamble barriers + weight/x0 transfers complete
    junk = const_pool.tile([P, 512], BF16)
    nc.vector.memset(junk[:], 0.0)
    for j in range(N_WARMUP):
        ps = ps_pool.tile([P, 512], F32, tag="ps", name=f"warm{j}")
        nc.tensor.matmul(
            ps[:], lhsT=junk[:, :P], rhs=junk[:], start=True, stop=True
        )

    for m in range(M_TILES):
        if m == 0:
            x_sb = xt0
        else:
            x_sb = x_pool.tile([P, D_IN], BF16, tag="xt", name=f"xt{m}")
            nc.sync.dma_start(x_sb[:], xt[ts(m, P), :])

        o_sb = o_pool.tile([P, D_OUT], BF16, tag="o")
        for h in range(NH):
            ps = ps_pool.tile([P, 512], F32, tag="ps")
            for k in range(K_TILES):
                nc.tensor.matmul(
                    ps[:],
                    lhsT=x_sb[:, ts(k, P)],
                    rhs=w_sb[:, ts(2 * k + h, 512)],
                    start=(k == 0),
                    stop=(k == K_TILES - 1),
                )
            # psum fp32 -> sbuf bf16; alternate engines so each half's
            # drain overlaps the other half's matmuls
            if h == 0:
                nc.scalar.copy(o_sb[:, ts(h, 512)], ps[:])
            else:
                nc.vector.tensor_copy(o_sb[:, ts(h, 512)], ps[:])
            nc.sync.dma_start(out[ts(m, P), ts(h, 512)], o_sb[:, ts(h, 512)])


def build_nc() -> "bass.Bass":
    nc = bacc.Bacc(
        "TRN2",
        target_bir_lowering=False,
        debug=False,
        num_devices=N_CORES,
    )
    xt = nc.dram_tensor("xt", [M_CORE, D_IN], BF16, kind="ExternalInput").ap()
    wp = nc.dram_tensor(
        "wp", [P, K_TILES * D_OUT], BF16, kind="ExternalInput"
    ).ap()
    out = nc.dram_tensor("out", [M_CORE, D_OUT], BF16, kind="ExternalOutput").ap()

    with tile.TileContext(nc) as tc, ExitStack() as ctx:
        _kernel_body(ctx, tc, xt, wp, out)
    nc.compile()
    return nc


_NC_CACHE: list = []


def get_nc() -> "bass.Bass":
    if not _NC_CACHE:
        _NC_CACHE.append(build_nc())
    return _NC_CACHE[0]


def make_in_maps(x, weight, a_w, b_w, magnitude):
    # derived DoRA weight, fully on host (tiny: 1024x1024)
    w = weight.astype(np.float32, copy=False)
    w_eff = w + SCALING * (b_w.astype(np.float32) @ a_w.astype(np.float32))
    norm = np.sqrt((w_eff.astype(np.float64) ** 2).sum(axis=1))
    s = (magnitude.reshape(-1) / norm).astype(np.float32)
    wT = np.ascontiguousarray((s[:, None] * w_eff).T)  # [d_in, d_out] f32
    # pack strips side by side: wp[q, k*1024 + n] = wT[k*128 + q, n]
    wp = (
        wT.reshape(K_TILES, P, D_OUT)
        .transpose(1, 0, 2)
        .reshape(P, K_TILES * D_OUT)
        .astype(NP_BF16)
    )

    # x: per-core PE-ready tiles; tile m holds [q, k*128 + t] =
    # x[m*128 + t, k*128 + q] so each k-slice is a matmul lhsT
    xf = x.reshape(M_TOT, D_IN).astype(np.float32, copy=False)
    in_maps = []
    for c in range(N_CORES):
        xc = xf[c * M_CORE : (c + 1) * M_CORE]
        ht = xc.reshape(M_TILES, P, K_TILES, P).transpose(0, 3, 2, 1)
        xt = np.ascontiguousarray(ht.astype(NP_BF16)).reshape(M_CORE, D_IN)
        in_maps.append({"xt": xt, "wp": wp})
    return in_maps


def kernel(x, weight, a_w, b_w, magnitude):
    nc = get_nc()
    in_maps = make_in_maps(x, weight, a_w, b_w, magnitude)
    trace = os.environ.get("KERNEL_TRACE", "0") == "1"
    res = run_bass_kernel_spmd(nc, in_maps, list(range(N_CORES)), trace=trace)
    if trace:
        kernel.last_result = res
    outs = [res.results[i]["out"].astype(np.float32) for i in range(N_CORES)]
    return np.concatenate(outs, axis=0).reshape(B, S, D_OUT)


# revision 11
# speedup vs baseline: 1.0066x; 1.0066x over previous
"""Trainium2 Bass kernel for nn_LoRALinear (DoRA-style LoRA linear).

Reference math (per problem):
    base = x @ W^T
    lora = sc * (x @ A^T) @ B^T          (sc = 2.0)
    w_eff = W + sc * (B @ A)
    s = magnitude / ||w_eff||_row         (row norm over in_dim)
    out = base + (s - 1) * base + s * lora
        = s * (base + lora)
        = x @ (s[:, None] * w_eff)^T

The whole op collapses to one dense matmul with a derived weight. The
derived weight is tiny (1024x1024) so it is computed on the host in fp32
and shipped per-core as bf16; the device does nothing but the big GEMM.

Sharding: data-parallel over batch*seq across 8 cores (4096 tokens each).

Layouts (host-prepared so the device never transposes):
  xt:  per core, token tile m (128 tokens) stored as [128 q, 8*128 (k,t)]
       with q = d_in within k-strip, t = token within tile. Each k-slice
       [:, k*128:(k+1)*128] is directly the stationary lhsT of a matmul.
  wp:  packed weight [128 q, 8*1024 (k,n)]: wp[q, k*1024+n] =
       wT[k*128+q, n] — one 2 MB DMA, each (k,h) slice is a moving rhs.
  out: [tokens, d_out] bf16, PSUM fp32 accumulate, converted on drain.

Per-core schedule:
  ~24 junk matmuls on a memset tile warm the PE p-state while the
  weight/x0 DMAs land (the PE runs at half speed until ~3us of
  continuous busy, and the DMAs need ~6us after the kernel preamble)
  main loop over 32 token tiles:
    DMA xt tile (256 KB, packets spread over all 16 DMA engines)
    2 psum halves x 8 k matmuls (bf16, 512-row moving dim, 1 cyc/row)
    drain half 0 on ACT, half 1 on DVE (psum fp32 -> sbuf bf16)
    DMA each half out as soon as its drain lands (shorter last-tile tail)
"""

import os
import numpy as np
import ml_dtypes
from contextlib import ExitStack

import concourse.bass as bass
import concourse.mybir as mybir
import concourse.tile as tile
from concourse import bacc
from concourse.bass import ts
from concourse.bass_utils import run_bass_kernel_spmd

N_CORES = 8
B, S, D_IN, D_OUT, R = 4, 8192, 1024, 1024, 16
SCALING = 32.0 / 16.0
M_TOT = B * S                 # 32768 tokens
M_CORE = M_TOT // N_CORES     # 4096 tokens per core
P = 128
M_TILES = M_CORE // P         # 32
K_TILES = D_IN // P           # 8
NH = D_OUT // 512             # 2 n-halves of 512
N_WARMUP = 6                  # junk matmuls to ramp the PE p-state
F32 = mybir.dt.float32
BF16 = mybir.dt.bfloat16
NP_BF16 = ml_dtypes.bfloat16


def _kernel_body(ctx: ExitStack, tc: "tile.TileContext", xt, wa, wb, out):
    nc = tc.nc
    const_pool = ctx.enter_context(tc.tile_pool(name="const", bufs=1))
    w_pool = ctx.enter_context(tc.tile_pool(name="w", bufs=1))
    x_pool = ctx.enter_context(tc.tile_pool(name="x", bufs=4))
    o_pool = ctx.enter_context(tc.tile_pool(name="o", bufs=4))
    ps_pool = ctx.enter_context(tc.tile_pool(name="ps", bufs=8, space="PSUM"))

    # startup transfers ride three different DMA queues so they overlap:
    # x0 on SP, the h=0 weight half on ACT, the h=1 half on GpSimd. The
    # first matmul is gated on x0 + wa only (~1.25 MB), not the full 2 MB.
    xt0 = x_pool.tile([P, D_IN], BF16, tag="xt", name="xt0")
    nc.sync.dma_start(xt0[:], xt[ts(0, P), :])

    wa_sb = w_pool.tile([P, K_TILES * 512], BF16, tag="wa", name="wa")
    nc.scalar.dma_start(wa_sb[:], wa[:, :])
    wb_sb = w_pool.tile([P, K_TILES * 512], BF16, tag="wb", name="wb")
    nc.gpsimd.dma_start(wb_sb[:], wb[:, :])
    w_sb = [wa_sb, wb_sb]

    # PE p-state warmup: junk matmuls with no DMA dependency fill the PE
    # while the preamble barriers + weight/x0 transfers complete
    junk = const_pool.tile([P, 512], BF16)
    nc.gpsimd.memset(junk[:], 0.0)
    for j in range(N_WARMUP):
        ps = ps_pool.tile([P, 512], F32, tag="ps", name=f"warm{j}")
        nc.tensor.matmul(
            ps[:], lhsT=junk[:, :P], rhs=junk[:], start=True, stop=True
        )

    for m in range(M_TILES):
        if m == 0:
            x_sb = xt0
        else:
            x_sb = x_pool.tile([P, D_IN], BF16, tag="xt", name=f"xt{m}")
            nc.sync.dma_start(x_sb[:], xt[ts(m, P), :])

        o_sb = o_pool.tile([P, D_OUT], BF16, tag="o")
        for h in range(NH):
            ps = ps_pool.tile([P, 512], F32, tag="ps")
            for k in range(K_TILES):
                nc.tensor.matmul(
                    ps[:],
                    lhsT=x_sb[:, ts(k, P)],
                    rhs=w_sb[h][:, ts(k, 512)],
                    start=(k == 0),
                    stop=(k == K_TILES - 1),
                )
            # psum fp32 -> sbuf bf16; alternate engines so each half's
            # drain overlaps the other half's matmuls
            if h == 0:
                nc.scalar.copy(o_sb[:, ts(h, 512)], ps[:])
            else:
                nc.vector.tensor_copy(o_sb[:, ts(h, 512)], ps[:])
            nc.sync.dma_start(out[ts(m, P), ts(h, 512)], o_sb[:, ts(h, 512)])


def build_nc() -> "bass.Bass":
    nc = bacc.Bacc(
        "TRN2",
        target_bir_lowering=False,
        debug=False,
        num_devices=N_CORES,
    )
    xt = nc.dram_tensor("xt", [M_CORE, D_IN], BF16, kind="ExternalInput").ap()
    wa = nc.dram_tensor("wa", [P, K_TILES * 512], BF16, kind="ExternalInput").ap()
    wb = nc.dram_tensor("wb", [P, K_TILES * 512], BF16, kind="ExternalInput").ap()
    out = nc.dram_tensor("out", [M_CORE, D_OUT], BF16, kind="ExternalOutput").ap()

    with tile.TileContext(nc) as tc, ExitStack() as ctx:
        _kernel_body(ctx, tc, xt, wa, wb, out)
    nc.compile()
    return nc


_NC_CACHE: list = []


def get_nc() -> "bass.Bass":
    if not _NC_CACHE:
        _NC_CACHE.append(build_nc())
    return _NC_CACHE[0]


def make_in_maps(x, weight, a_w, b_w, magnitude):
    # derived DoRA weight, fully on host (tiny: 1024x1024)
    w = weight.astype(np.float32, copy=False)
    w_eff = w + SCALING * (b_w.astype(np.float32) @ a_w.astype(np.float32))
    norm = np.sqrt((w_eff.astype(np.float64) ** 2).sum(axis=1))
    s = (magnitude.reshape(-1) / norm).astype(np.float32)
    wT = np.ascontiguousarray((s[:, None] * w_eff).T)  # [d_in, d_out] f32
    # pack per n-half, strips side by side: wa[q, k*512 + n] =
    # wT[k*128 + q, n], wb the same for n >= 512
    wkqn = wT.reshape(K_TILES, P, D_OUT).transpose(1, 0, 2)  # [q, k, n]
    wa = np.ascontiguousarray(wkqn[:, :, :512]).reshape(P, K_TILES * 512).astype(NP_BF16)
    wb = np.ascontiguousarray(wkqn[:, :, 512:]).reshape(P, K_TILES * 512).astype(NP_BF16)

    # x: per-core PE-ready tiles; tile m holds [q, k*128 + t] =
    # x[m*128 + t, k*128 + q] so each k-slice is a matmul lhsT
    xf = x.reshape(M_TOT, D_IN).astype(np.float32, copy=False)
    in_maps = []
    for c in range(N_CORES):
        xc = xf[c * M_CORE : (c + 1) * M_CORE]
        ht = xc.reshape(M_TILES, P, K_TILES, P).transpose(0, 3, 2, 1)
        xt = np.ascontiguousarray(ht.astype(NP_BF16)).reshape(M_CORE, D_IN)
        in_maps.append({"xt": xt, "wa": wa, "wb": wb})
    return in_maps


def kernel(x, weight, a_w, b_w, magnitude):
    nc = get_nc()
    in_maps = make_in_maps(x, weight, a_w, b_w, magnitude)
    trace = os.environ.get("KERNEL_TRACE", "0") == "1"
    res = run_bass_kernel_spmd(nc, in_maps, list(range(N_CORES)), trace=trace)
    if trace:
        kernel.last_result = res
    outs = [res.results[i]["out"].astype(np.float32) for i in range(N_CORES)]
    return np.concatenate(outs, axis=0).reshape(B, S, D_OUT)
